# revision 13
# baseline (speedup 1.0000x reference)
"""MoE grouped-FFN kernel for Trainium2 (8 NeuronCores, expert-parallel).

Problem: x [1, 2048, 1024] fp32, 32 experts x 64 tokens each,
per-expert FFN 1024 -> 4096 (gelu) -> 1024.

Sharding: expert-parallel, 4 experts per core. Tokens are statically
pre-chunked per expert (dim 1 == E*C), so each core just gets its 4
experts' token rows + weights; outputs concatenate back. No collectives.

Weights/activations stream in bf16 (the problem is HBM-bound; halves
traffic vs fp32, measured end-to-end rel err ~3e-3 vs the 2e-2 gate);
PSUM accumulation stays fp32.

Per-core dataflow:
  mm1: h[c,512f] += xT[k].T @ W1[k, fchunk]   (tokens on M=64, K-accum in PSUM)
  DVE: hs = bf16(h_psum); PE-transpose hs -> hT [f, c]
  ACT: hT_sb = gelu(hT + b1) (per-partition bias)
  mm2: out[c,512d] += hT[ft].T @ W2[ft, dchunk]
  DVE: out_sb = out_psum + b2  (evacuation fused with bias add)
"""

import os
import numpy as np

E, C, D, F = 32, 64, 1024, 4096
N_CORES = 8
E_LOC = E // N_CORES  # experts per core
P = 128
NMAX = 512  # matmul moving-operand max for 4-byte dtypes
KT1 = D // P  # 8 K-tiles in mm1
FT = F // P  # 32 f-tiles (contraction tiles for mm2)

# Tuning knobs (SBUF budget is ~192KB/partition under Tile).
DEFAULT_CFG = dict(
    w1_chunk=512,  # F columns per W1 DMA (x8 KT1 rows -> chunk/128 MB)
    w2_block=4,    # f-tiles per W2 DMA (x D columns)
    w1_bufs=4,
    w2_bufs=3,
    hs_bufs=8,
    ht_bufs=2,
    os_bufs=2,
    lag=0,         # emit transposes of chunk c after mm1 of chunk c+1
    # split_engines adds gpsimd (SWDGE) as a third DMA issue path; measured
    # ~equal within noise (382-428us band) and less battle-tested, so off.
    split_engines=False,
)

_CACHE = {}
LAST_RESULTS = None  # BassKernelResults of the most recent run (for profiling)


def _build_program(act="gelu", repeats=1, cfg=None):
    import contextlib

    import concourse.bacc as bacc
    import concourse.tile as tile
    import concourse.mybir as mybir

    cfg = dict(DEFAULT_CFG, **(cfg or {}))

    f32 = mybir.dt.float32
    bf16 = mybir.dt.bfloat16  # streamed operand dtype; PSUM accum stays fp32
    # CoreSim doesn't implement the Gelu LUTs; "tanh" is a sim-only stand-in
    # used by test.py to validate everything except the activation itself.
    GELU = {
        "gelu": mybir.ActivationFunctionType.Gelu_apprx_tanh,
        "tanh": mybir.ActivationFunctionType.Tanh,
    }[act]
    ADD = mybir.AluOpType.add

    nc = bacc.Bacc("TRN2", target_bir_lowering=False, debug=False)

    n_ch = F // cfg["w1_chunk"]
    n_fb = FT // cfg["w2_block"]
    xT_d = nc.declare_dram_parameter("xT", [P, E_LOC, KT1, C], bf16, isOutput=False)
    # w1/w2 arrive host-pre-tiled so every weight DMA is one contiguous read:
    # w1[e, c, p, k, fc] = W1[e, k*128+p, c*chunk+fc]
    # w2[e, fb, p, j, d] = W2[e, (fb*block+j)*128+p, d]
    w1_d = nc.declare_dram_parameter(
        "w1", [E_LOC, n_ch, P, KT1, cfg["w1_chunk"]], bf16, isOutput=False
    )
    w2_d = nc.declare_dram_parameter(
        "w2", [E_LOC, n_fb, P, cfg["w2_block"], D], bf16, isOutput=False
    )
    b1_d = nc.declare_dram_parameter("b1t", [P, E_LOC, FT], f32, isOutput=False)
    b2_d = nc.declare_dram_parameter("b2r", [C, E_LOC, D], f32, isOutput=False)
    id_d = nc.declare_dram_parameter("ident", [C, C], bf16, isOutput=False)
    out_d = nc.declare_dram_parameter("out", [E_LOC * C, D], f32, isOutput=True)

    w1_ap = w1_d.ap()  # [e, chunk, 128, KT1, w1_chunk]
    w2_ap = w2_d.ap()  # [e, fblock, 128, w2_block, D]

    with tile.TileContext(nc) as tc:
        with (
            tc.tile_pool(name="const", bufs=1) as const_pool,
            tc.tile_pool(name="w1", bufs=cfg["w1_bufs"]) as w1_pool,
            tc.tile_pool(name="w2", bufs=cfg["w2_bufs"]) as w2_pool,
            tc.tile_pool(name="hs", bufs=cfg["hs_bufs"]) as hs_pool,
            tc.tile_pool(name="ht", bufs=cfg["ht_bufs"]) as ht_pool,
            tc.tile_pool(name="os", bufs=cfg["os_bufs"]) as os_pool,
            tc.tile_pool(name="ph", bufs=2, space="PSUM") as ph_pool,
            tc.tile_pool(name="pt", bufs=4, space="PSUM") as pt_pool,
            tc.tile_pool(name="po", bufs=2, space="PSUM") as po_pool,
        ):
            pools = dict(
                w1=w1_pool, w2=w2_pool, hs=hs_pool, ht=ht_pool, os=os_pool,
                ph=ph_pool, pt=pt_pool, po=po_pool,
            )
            xT_sb = const_pool.tile([P, E_LOC, KT1, C], bf16, tag="xt")
            nc.sync.dma_start(out=xT_sb, in_=xT_d.ap())
            b1_sb = const_pool.tile([P, E_LOC, FT], f32, tag="b1")
            nc.sync.dma_start(out=b1_sb, in_=b1_d.ap())
            b2_sb = const_pool.tile([C, E_LOC, D], f32, tag="b2")
            nc.sync.dma_start(out=b2_sb, in_=b2_d.ap())
            id_sb = const_pool.tile([C, C], bf16, tag="id")
            nc.sync.dma_start(out=id_sb, in_=id_d.ap())

            consts = (xT_sb, b1_sb, b2_sb, id_sb)

            # repeats>1 wraps the computation in a hardware loop so a single
            # execute measures R back-to-back runs (benchmarking only).
            rep_ctx = (
                tc.For_i(0, repeats, 1) if repeats > 1 else contextlib.nullcontext()
            )
            with rep_ctx:
                _emit_body(
                    nc, GELU, ADD, consts, w1_ap, w2_ap, out_d, pools,
                    f32, bf16, cfg,
                )

    nc.compile()
    return nc


def _emit_body(nc, GELU, ADD, consts, w1_ap, w2_ap, out_d, pools, f32, bf16, cfg):
    xT_sb, b1_sb, b2_sb, id_sb = consts
    w1_chunk = cfg["w1_chunk"]
    w2_block = cfg["w2_block"]
    lag = cfg["lag"]
    n_chunks = F // w1_chunk
    sub = w1_chunk // NMAX  # 512-wide sub-chunks per W1 DMA

    for e in range(E_LOC):
        # ---- phase 1: h = x_e @ W1_e (pre-activation, tokens on M)
        pending = []  # (chunk_idx, [hs tiles]) awaiting transpose

        def flush_transposes(hT):
            while pending:
                ci, hs_list = pending.pop(0)
                for s, hs in enumerate(hs_list):
                    for t in range(NMAX // P):
                        ft = (ci * sub + s) * (NMAX // P) + t
                        tp = pools["pt"].tile([P, C], bf16, tag="tp")
                        nc.tensor.transpose(
                            tp, in_=hs[:, t * P : (t + 1) * P], identity=id_sb
                        )
                        nc.scalar.activation(
                            out=hT[:, ft, :],
                            in_=tp,
                            func=GELU,
                            bias=b1_sb[:, e, ft : ft + 1],
                        )

        hT = pools["ht"].tile([P, FT, C], bf16, tag="ht")
        for c in range(n_chunks):
            w1t = pools["w1"].tile([P, KT1, w1_chunk], bf16, tag="w1t")
            if cfg.get("rr3"):
                w1_eng = (nc.sync, nc.scalar, nc.gpsimd)[c % 3]
            elif cfg.get("split_engines"):
                w1_eng = (nc.sync, nc.gpsimd)[c % 2]
            else:
                w1_eng = nc.sync
            w1_eng.dma_start(out=w1t, in_=w1_ap[e, c])
            hs_list = []
            for s in range(sub):
                hp = pools["ph"].tile([C, NMAX], f32, tag="hp")
                for k in range(KT1):
                    nc.tensor.matmul(
                        hp,
                        lhsT=xT_sb[:, e, k, :],
                        rhs=w1t[:, k, s * NMAX : (s + 1) * NMAX],
                        start=(k == 0),
                        stop=(k == KT1 - 1),
                    )
                hs = pools["hs"].tile([C, NMAX], bf16, tag="hs")
                nc.vector.tensor_copy(out=hs, in_=hp)
                hs_list.append(hs)
            pending.append((c, hs_list))
            if len(pending) > lag:
                flush_transposes(hT)
        flush_transposes(hT)

        # ---- phase 2: out_e = gelu(h) @ W2_e + b2_e
        op0 = pools["po"].tile([C, NMAX], f32, tag="op")
        op1 = pools["po"].tile([C, NMAX], f32, tag="op")
        for fb in range(FT // w2_block):
            w2t = pools["w2"].tile([P, w2_block, D], bf16, tag="w2t")
            if cfg.get("rr3"):
                w2_eng = (nc.gpsimd, nc.sync, nc.scalar)[fb % 3]
            elif cfg.get("split_engines"):
                w2_eng = (nc.scalar, nc.gpsimd)[fb % 2]
            else:
                w2_eng = nc.scalar
            w2_eng.dma_start(out=w2t, in_=w2_ap[e, fb])
            for j in range(w2_block):
                ft = w2_block * fb + j
                nc.tensor.matmul(
                    op0,
                    lhsT=hT[:, ft, :],
                    rhs=w2t[:, j, 0:NMAX],
                    start=(ft == 0),
                    stop=(ft == FT - 1),
                )
                nc.tensor.matmul(
                    op1,
                    lhsT=hT[:, ft, :],
                    rhs=w2t[:, j, NMAX:D],
                    start=(ft == 0),
                    stop=(ft == FT - 1),
                )
        os_t = pools["os"].tile([C, D], f32, tag="os")
        nc.vector.tensor_tensor(os_t[:, 0:NMAX], op0, b2_sb[:, e, 0:NMAX], ADD)
        nc.vector.tensor_tensor(os_t[:, NMAX:D], op1, b2_sb[:, e, NMAX:D], ADD)
        nc.sync.dma_start(out=out_d.ap()[e * C : (e + 1) * C, :], in_=os_t)


def _get_program(act="gelu", repeats=1, cfg=None):
    key = (act, repeats, tuple(sorted((cfg or {}).items())))
    if key not in _CACHE:
        _CACHE[key] = _build_program(act, repeats, cfg)
    return _CACHE[key]


def make_in_maps(x, W1, b1, W2, b2):
    import ml_dtypes

    bf16 = ml_dtypes.bfloat16
    x = np.ascontiguousarray(np.asarray(x, dtype=np.float32))
    W1 = np.asarray(W1, dtype=np.float32).astype(bf16)
    b1 = np.ascontiguousarray(np.asarray(b1, dtype=np.float32))
    W2 = np.asarray(W2, dtype=np.float32).astype(bf16)
    b2 = np.ascontiguousarray(np.asarray(b2, dtype=np.float32))
    ident = np.eye(C, dtype=bf16)
    in_maps = []
    for i in range(N_CORES):
        lo, hi = i * E_LOC, (i + 1) * E_LOC
        xc = x[0, lo * C : hi * C, :].reshape(E_LOC, C, KT1, P)
        xT = np.ascontiguousarray(xc.transpose(3, 0, 2, 1)).astype(bf16)  # [128,e,k,c]
        b1t = np.ascontiguousarray(
            b1[lo:hi].reshape(E_LOC, FT, P).transpose(2, 0, 1)
        )  # [128, e, ft]
        b2r = np.ascontiguousarray(
            np.broadcast_to(b2[lo:hi][None], (C, E_LOC, D))
        )  # [64, e, d]
        n_ch = F // DEFAULT_CFG["w1_chunk"]
        wb = DEFAULT_CFG["w2_block"]
        w1p = np.ascontiguousarray(
            W1[lo:hi].reshape(E_LOC, KT1, P, n_ch, DEFAULT_CFG["w1_chunk"])
            .transpose(0, 3, 2, 1, 4)
        )
        w2p = np.ascontiguousarray(
            W2[lo:hi].reshape(E_LOC, FT // wb, wb, P, D).transpose(0, 1, 3, 2, 4)
        )
        in_maps.append(
            {
                "xT": xT,
                "w1": w1p,
                "w2": w2p,
                "b1t": b1t,
                "b2r": b2r,
                "ident": ident,
            }
        )
    return in_maps


def kernel(x, W1, b1, W2, b2):
    global LAST_RESULTS
    from concourse.bass_utils import run_bass_kernel_spmd

    nc = _get_program()
    in_maps = make_in_maps(x, W1, b1, W2, b2)
    trace = bool(int(os.environ.get("KERNEL_TRACE", "0")))
    res = run_bass_kernel_spmd(nc, in_maps, list(range(N_CORES)), trace=trace)
    LAST_RESULTS = res
    out = np.concatenate([r["out"] for r in res.results], axis=0)
    return out.reshape(1, E * C, D).astype(np.float32)

